# revision 54
# baseline (speedup 1.0000x reference)
"""GAT (2-layer graph attention network) on 8 Trainium2 NeuronCores.

Strategy (1D node partition):
  - Each core owns R = N/8 rows (nodes) of the attention matrix.
  - Exp-free scores: softmax over column i is invariant to any per-i scale,
    so e^{lrelu(s1_i+s2_j)} / e^{s1_i} = max(q_j, w_i*u_j) with per-node
    q = e^{s2}, u = e^{0.2 s2}, w = e^{-0.8 s1}.  The N^2 path is then a
    single 4x-mode tensor_scalar (mult+max) plus one batched tensor_tensor
    multiply with a {0,1} mask tile (exact: masked z becomes exactly 0).
    No activation-engine pass over N^2 elements at all.
  - Aggregation is "flipped": z-blocks [j,128i] are the PE stationary and
    [Wh_h | ones] streams 129 columns -> out[i, 129] accumulated over j in
    PSUM; column 128 is the softmax denominator for free (no separate
    ones-vector matmuls).
  - AllGather payload per node: 4x(Wh_h|1) + q,u per head (pre-exponentiated
    locally) = 524 cols; no replicated X compute, no full-N projections.
  - Layer 2 runs the same scheme with payload [Wh2|1|q2|u2].
  - PSUM note: matmul start=True resets the whole 2KB bank, so of the 16
    concurrently-open accumulation chains only the first chain per bank
    issues start=True; the rest accumulate from the bank-wide zero (PE
    executes in program order, making this safe).
"""

import math
from contextlib import ExitStack
from dataclasses import dataclass

import numpy as np
import ml_dtypes

import concourse.bass as bass
import concourse.mybir as mybir
import concourse.tile as tile
from concourse import bacc
from concourse.bass_utils import run_bass_kernel_spmd

BF16 = ml_dtypes.bfloat16
ALPHA = 0.2
MASK_K = 1.0

# --------------------------------------------------------------------------
# Custom fused DVE op (elu select), registered into concourse.dve_ops
# --------------------------------------------------------------------------

import concourse.dve_ops as dve_ops
from concourse.dve_spec import (
    Spec, Src0, Src1, C0, Zero, lower, select, _has_src1,
)
from concourse.dve_uop import DveOpSpec


def _make_elu_spec():
    # out = in0 > 0 ? in0 : in1 - s0   (elu with in1 = exp(in0), s0 = 1.0)
    def _elu_ref(in0, in1, s0, s1, imm2):
        x = in0.astype(np.float32)
        return np.where(x > 0, x, in1.astype(np.float32) - s0)

    return Spec(body=select(Src0 > Zero, Src0, Src1 - C0), reference=_elu_ref)


def _register(name, spec):
    if name in dve_ops._SUB_OPCODE_FOR_NAME:
        for op in dve_ops.OPS:
            if op.name == name:
                return op
    row = max(dve_ops._SUB_OPCODE_FOR_NAME.values()) + 1
    assert row < 0x20
    shas = {}
    for ver in ("v3", "v4"):
        uops = lower(spec, ver=ver)
        shas[ver] = DveOpSpec(
            name=name, opcode=row, uops=uops, rd1_en=_has_src1(spec)
        ).sha(ver)
    op = dve_ops.DveOp(name, spec, subdim=False, uops_sha=shas)
    dve_ops.OPS.append(op)
    dve_ops.CUSTOM_DVE_SPECS[name] = spec
    dve_ops._SUB_OPCODE_FOR_NAME[name] = row
    return op


ELU_SEL = _register("ELU_SEL_GAT", _make_elu_spec())


# --------------------------------------------------------------------------
# Kernel configuration
# --------------------------------------------------------------------------

@dataclass(frozen=True)
class Cfg:
    N: int = 4096      # nodes
    C: int = 512       # input feature dim
    H: int = 128       # hidden per head (must be 128)
    HEADS: int = 4
    F2: int = 64       # output dim
    CORES: int = 8
    GRP: int = 8       # j-tiles per batched mask-min

    @property
    def R(self): return self.N // self.CORES          # rows per core
    @property
    def JT(self): return self.N // 128                # j tiles
    @property
    def CT(self): return self.C // 128                # input-feature tiles
    @property
    def HH(self): return self.HEADS * self.H          # layer-1 out features
    @property
    def CT2(self): return self.HH // 128              # layer-2 contraction tiles
    @property
    def RT(self): return self.R // 128                # local row tiles
    @property
    def PW1(self): return self.HEADS * 129 + 2 * self.HEADS  # L1 payload cols
    @property
    def PAY2(self): return self.F2 + 3                # L2 payload cols
    @property
    def NG(self): return self.JT // self.GRP


FULL = Cfg()


# --------------------------------------------------------------------------
# Device program
# --------------------------------------------------------------------------

def build_gat_nc(cfg: Cfg, collective: bool = True, iters: int = 1,
                 loop_iters: int = 0, phases: str = "full",
                 pool_den: int = 8, pool_num: int = 0, dump: str = ""):
    dt = mybir.dt.bfloat16
    f32 = mybir.dt.float32
    add = mybir.AluOpType.add
    mult = mybir.AluOpType.mult
    mx = mybir.AluOpType.max
    mn = mybir.AluOpType.min
    bypass = mybir.AluOpType.bypass
    Exp = mybir.ActivationFunctionType.Exp
    Copy = mybir.ActivationFunctionType.Copy

    N, C, HEADS, F2, R = cfg.N, cfg.C, cfg.HEADS, cfg.F2, cfg.R
    JT, CT, HH, CT2, RT = cfg.JT, cfg.CT, cfg.HH, cfg.CT2, cfg.RT
    GRP, NG, PW1, PAY2 = cfg.GRP, cfg.NG, cfg.PW1, cfg.PAY2
    F2p = F2 + 2
    QOFF = HEADS * 129            # offset of q cols within a payload block
    UOFF = QOFF + HEADS           # offset of u cols

    nc = bacc.Bacc(
        "TRN2", target_bir_lowering=False, debug=False, num_devices=cfg.CORES
    )

    # ---- DRAM I/O -------------------------------------------------------
    xtl_d = nc.dram_tensor("xtloc", [128, CT * R], dt, kind="ExternalInput").ap()
    km_d = nc.dram_tensor("km", [128, JT * R], dt, kind="ExternalInput").ap()
    w1c_d = nc.dram_tensor("w1c", [128, CT * HH], dt, kind="ExternalInput").ap()
    ws1_d = nc.dram_tensor("ws1", [128, CT * 8], dt, kind="ExternalInput").ap()
    w2a_d = nc.dram_tensor("w2a", [128, CT2 * F2p], dt, kind="ExternalInput").ap()
    id_d = nc.dram_tensor("ident", [128, 128], dt, kind="ExternalInput").ap()
    out_d = nc.dram_tensor("out", [R, F2], f32, kind="ExternalOutput").ap()

    with tile.TileContext(nc) as tc, ExitStack() as ctx:
        const = ctx.enter_context(tc.tile_pool(name="const", bufs=1))
        work = ctx.enter_context(tc.tile_pool(name="work", bufs=4))
        wz = ctx.enter_context(tc.tile_pool(name="wz", bufs=4))
        psn = ctx.enter_context(tc.tile_pool(name="psn", bufs=1, space="PSUM"))
        pstp = ctx.enter_context(tc.tile_pool(name="pstp", bufs=2, space="PSUM"))
        dram = ctx.enter_context(tc.tile_pool(name="dram", bufs=1, space="DRAM"))

        whsend_t = dram.tile([128, RT * PW1], dt)
        gsend_t = dram.tile([128, RT * PAY2], dt)
        if cfg.CORES > 4:
            whfull_t = nc.dram_tensor(
                "whfull_sh", [cfg.CORES * 128, RT * PW1], dt,
                addr_space="Shared").ap()
            gfull_t = nc.dram_tensor(
                "gfull_sh", [cfg.CORES * 128, RT * PAY2], dt,
                addr_space="Shared").ap()
        else:
            whfull_t = dram.tile([cfg.CORES * 128, RT * PW1], dt)
            gfull_t = dram.tile([cfg.CORES * 128, RT * PAY2], dt)

        import contextlib
        loop_cm = (tc.For_i(0, loop_iters, 1) if loop_iters
                   else contextlib.nullcontext())
        with loop_cm:
          for _it in range(iters):
            # ---- constant loads ---------------------------------------
            _half = CT * R // 2
            xtl_sb = const.tile([128, CT * R], dt)
            nc.sync.dma_start(out=xtl_sb[:, 0:_half], in_=xtl_d[:, 0:_half])
            nc.scalar.dma_start(out=xtl_sb[:, _half:], in_=xtl_d[:, _half:])
            ws1_sb = const.tile([128, CT * 8], dt)
            nc.sync.dma_start(out=ws1_sb, in_=ws1_d)
            identb = const.tile([128, 128], dt)
            nc.sync.dma_start(out=identb, in_=id_d)
            _hw = CT * HH // 2
            w1c_sb = const.tile([128, CT * HH], dt)
            nc.sync.dma_start(out=w1c_sb[:, 0:_hw], in_=w1c_d[:, 0:_hw])
            nc.scalar.dma_start(out=w1c_sb[:, _hw:], in_=w1c_d[:, _hw:])
            w2a_sb = const.tile([128, CT2 * F2p], dt)
            nc.scalar.dma_start(out=w2a_sb, in_=w2a_d)
            km_sb = const.tile([128, JT * R], dt)
            _hm = JT * R // 8

            def km_load(q, eng):
                eng.dma_start(
                    out=km_sb[:, q * _hm: (q + 1) * _hm],
                    in_=km_d[:, q * _hm: (q + 1) * _hm])

            for _q in range(3):
                km_load(_q, nc.scalar)

            if phases == "dma":
                zf = const.tile([128, F2], f32)
                nc.vector.memset(zf, 0.0)
                for rt in range(RT):
                    nc.sync.dma_start(out=out_d[rt * 128:(rt + 1) * 128, :],
                                      in_=zf)
                continue

            # ---- phase A: projections first (unblocks wb), then Wh ----
            snd = const.tile([128, RT * PW1], dt)
            wr_sb = const.tile([1, HEADS * R], dt)
            for rt in range(RT):
                pSt = psn.tile([128, HH], f32, tag=("s1" if rt % 2 else "s0"),
                               name=f"pSt{rt}")
                pS = pSt[:, 0:8]
                for ct in range(CT):
                    nc.tensor.matmul(
                        out=pS,
                        lhsT=xtl_sb[:, ct * R + rt * 128: ct * R + (rt + 1) * 128],
                        rhs=ws1_sb[:, ct * 8: (ct + 1) * 8],
                        start=(ct == 0), stop=(ct == CT - 1),
                    )
                b = rt * PW1
                # q = exp(s2), u = exp(alpha*s2); s2 = proj cols 4..8
                nc.scalar.activation(
                    out=snd[:, b + QOFF: b + QOFF + HEADS],
                    in_=pS[:, 4:8], func=Exp)
                nc.scalar.activation(
                    out=snd[:, b + UOFF: b + UOFF + HEADS],
                    in_=pS[:, 4:8], func=Exp, scale=ALPHA)
                # w = exp(-0.8*s1) computed from f32 psum in column form,
                # then transposed to a row per head (partition-0 reads only)
                wcol = work.tile([128, 4], dt, tag="wcol")
                nc.scalar.activation(out=wcol, in_=pS[:, 0:4], func=Exp,
                                     scale=ALPHA - 1.0)
                for h in range(HEADS):
                    pT = pstp.tile([128, 128], dt, tag="tp")
                    nc.tensor.transpose(out=pT[0:1, :], in_=wcol[:, h: h + 1],
                                        identity=identb)
                    nc.vector.tensor_copy(
                        out=wr_sb[0:1, h * R + rt * 128: h * R + (rt + 1) * 128],
                        in_=pT[0:1, :])

            # broadcast w rows per head (early: gates first zr)
            wb = const.tile([128, HEADS * R], dt)
            for h in range(HEADS):
                nc.gpsimd.partition_broadcast(
                    out_ap=wb[:, h * R: (h + 1) * R],
                    in_ap=wr_sb[0:1, h * R: (h + 1) * R])

            for rt in range(RT):
                pA = psn.tile([128, HH], f32, tag=f"n{rt}", name=f"pA{rt}")
                for ct in range(CT):
                    nc.tensor.matmul(
                        out=pA,
                        lhsT=xtl_sb[:, ct * R + rt * 128: ct * R + (rt + 1) * 128],
                        rhs=w1c_sb[:, ct * HH: (ct + 1) * HH],
                        start=(ct == 0), stop=(ct == CT - 1),
                    )
                b = rt * PW1
                snd_wh = snd[:, b: b + HEADS * 129].rearrange(
                    "p (h c) -> p h c", c=129)[:, :, 0:128]
                pA_r = pA[:, :].rearrange("p (h c) -> p h c", c=128)
                if rt % 2 == 0:
                    nc.scalar.copy(out=snd_wh, in_=pA_r)
                else:
                    nc.vector.tensor_copy(out=snd_wh, in_=pA_r)
                nc.vector.memset(
                    snd[:, b: b + HEADS * 129].rearrange(
                        "p (h c) -> p h c", c=129)[:, :, 128:129], 1.0)
                nc.sync.dma_start(
                    out=whsend_t[:, b: b + PW1], in_=snd[:, b: b + PW1])

            # ---- gather Wh+q+u payload --------------------------------
            if collective:
                nc.gpsimd.collective_compute(
                    "AllGather", bypass,
                    replica_groups=[list(range(cfg.CORES))],
                    ins=[whsend_t.opt()], outs=[whfull_t.opt()],
                )
            wh_sb = const.tile([128, JT * PW1], dt)
            if collective:
                for c in range(cfg.CORES):
                    nc.sync.dma_start(
                        out=wh_sb[:, c * RT * PW1: (c + 1) * RT * PW1],
                        in_=whfull_t[c * 128: (c + 1) * 128, :])
                    if c < 5:
                        km_load(3 + c, nc.sync)
            else:
                for c in range(cfg.CORES):
                    nc.sync.dma_start(
                        out=whfull_t[c * 128: (c + 1) * 128, :],
                        in_=whsend_t[:, :])
                    nc.sync.dma_start(
                        out=wh_sb[:, c * RT * PW1: (c + 1) * RT * PW1],
                        in_=whfull_t[c * 128: (c + 1) * 128, :])
            # f32 copies of the q/u scalar columns (tensor_scalar needs f32);
            # emitted lazily inside the first head's group loop so each only
            # waits on its own core block of the gather
            quf = const.tile([128, JT * 2 * HEADS], f32)
            wh_r = wh_sb[:, :].rearrange("p (t c) -> p t c", c=PW1)
            quf_r = quf[:, :].rearrange("p (t c) -> p t c", c=2 * HEADS)

            def quf_copy(c):
                nc.vector.tensor_copy(
                    out=quf_r[:, c * RT: (c + 1) * RT, :],
                    in_=wh_r[:, c * RT: (c + 1) * RT, QOFF: QOFF + 2 * HEADS])

            if phases == "wh":
                zf = const.tile([128, F2], f32)
                nc.vector.memset(zf, 0.0)
                for rt in range(RT):
                    nc.sync.dma_start(out=out_d[rt * 128:(rt + 1) * 128, :],
                                      in_=zf)
                continue

            # ---- layer 1 attention (group-major: consume gather blocks
            #      as they arrive; all 4 heads accumulate concurrently) ----
            hlocT = const.tile([128, CT2 * R], dt)
            dmp = (const.tile([128, 256], f32, name="dmp")
                   if dump in ("z00", "num0") else None)
            # (h, it) -> psum range: h<3 -> tag n{it} @ h*129;
            # h==3 -> spill tags s0 (it<3 @ it*129) / s1 (it==3 @ 0)
            psNt = [psn.tile([128, HH], f32, tag=f"n{it}", name=f"psNt{it}")
                    for it in range(RT)]
            psS0 = psn.tile([128, HH], f32, tag="s0", name="psS0")
            psS1 = psn.tile([128, HH], f32, tag="s1", name="psS1")

            def psn_range(h, it):
                if h < 3:
                    return psNt[it], h * 129
                if it < 3:
                    return psS0, it * 129
                return psS1, 0
            for g in range(NG):
                quf_copy(2 * g)
                quf_copy(2 * g + 1)
                for h in range(HEADS):
                    zr = work.tile([128, GRP * R], dt, tag="zr")
                    for k in range(GRP):
                        t = g * GRP + k
                        base = t * 2 * HEADS
                        nc.vector.tensor_scalar(
                            out=zr[:, k * R: (k + 1) * R],
                            in0=wb[:, h * R: (h + 1) * R],
                            scalar1=quf[:, base + HEADS + h: base + HEADS + h + 1],
                            scalar2=quf[:, base + h: base + h + 1],
                            op0=mult, op1=mx,
                        )
                    zg = wz.tile([128, GRP * R], dt, tag="zg")
                    _sp = pool_num * R
                    base_m = g * GRP * R
                    nc.vector.tensor_tensor(
                        out=zg[:, 0: GRP * R - _sp], in0=zr[:, 0: GRP * R - _sp],
                        in1=km_sb[:, base_m: base_m + GRP * R - _sp], op=mult)
                    if _sp:
                        nc.gpsimd.tensor_tensor(
                            out=zg[:, GRP * R - _sp:], in0=zr[:, GRP * R - _sp:],
                            in1=km_sb[:, base_m + GRP * R - _sp: base_m + GRP * R],
                            op=mult)
                    if dump == "z00" and g == 0 and h == 0:
                        nc.vector.tensor_copy(out=dmp, in_=zg[:, 0:256])
                    for k in range(GRP):
                        t = g * GRP + k
                        for it in range(RT):
                            pt_, off_ = psn_range(h, it)
                            # start=True resets the WHOLE psum bank: only the
                            # first chain per bank may use it (h==0 zeroes
                            # n{it}; h==3 it==0 zeroes s0, it==3 zeroes s1)
                            first = (t == 0 and (h == 0 or
                                     (h == 3 and it in (0, 3))))
                            nc.tensor.matmul(
                                out=pt_[:, off_: off_ + 129],
                                lhsT=zg[:, k * R + it * 128: k * R + (it + 1) * 128],
                                rhs=wh_sb[:, t * PW1 + h * 129: t * PW1 + (h + 1) * 129],
                                start=first, stop=(t == JT - 1),
                            )
            if dump == "num0":
                nc.vector.tensor_copy(out=dmp[:, 0:129], in_=psNt[0][:, 0:129])
                nc.vector.memset(dmp[:, 129:256], 0.0)
            # normalize + elu + transpose into hlocT (it-major), then the
            # layer-2 projection for that row tile accumulates immediately
            gsnd = const.tile([128, RT * PAY2], dt)
            w2r_sb = const.tile([1, R], dt)
            for it in range(RT):
                for h in range(HEADS):
                    pt_, off_ = psn_range(h, it)
                    rcp = work.tile([128, 1], f32, tag="rcp")
                    nc.vector.reciprocal(
                        out=rcp, in_=pt_[:, off_ + 128: off_ + 129])
                    hni = work.tile([128, 128], dt, tag="hni")
                    nc.scalar.activation(out=hni, in_=pt_[:, off_: off_ + 128],
                                         func=Copy, scale=rcp)
                    ehi = work.tile([128, 128], dt, tag="ehi")
                    nc.scalar.activation(out=ehi, in_=hni, func=Exp)
                    helu = work.tile([128, 128], dt, tag="helu")
                    nc.vector._custom_dve(
                        ELU_SEL, out=helu, in0=hni, in1=ehi,
                        s0=1.0, s1=0.0, imm2=0.0,
                    )
                    pT2 = pstp.tile([128, 128], dt, tag="tp")
                    nc.tensor.transpose(out=pT2, in_=helu, identity=identb)
                    nc.scalar.copy(
                        out=hlocT[:, h * R + it * 128: h * R + (it + 1) * 128],
                        in_=pT2)
                # layer-2 projection for this row tile starts immediately
                pWt = psn.tile([128, HH], f32, tag=f"n{it}", name=f"pWt{it}")
                pW = pWt[:, 0:F2p]
                for ct2 in range(CT2):
                    nc.tensor.matmul(
                        out=pW,
                        lhsT=hlocT[:, ct2 * R + it * 128: ct2 * R + (it + 1) * 128],
                        rhs=w2a_sb[:, ct2 * F2p: (ct2 + 1) * F2p],
                        start=(ct2 == 0), stop=(ct2 == CT2 - 1),
                    )
                b2 = it * PAY2
                nc.scalar.copy(out=gsnd[:, b2: b2 + F2], in_=pW[:, 0:F2])
                nc.vector.memset(gsnd[:, b2 + F2: b2 + F2 + 1], 1.0)
                nc.scalar.activation(
                    out=gsnd[:, b2 + F2 + 1: b2 + F2 + 2],
                    in_=pW[:, F2 + 1: F2 + 2], func=Exp)
                nc.scalar.activation(
                    out=gsnd[:, b2 + F2 + 2: b2 + F2 + 3],
                    in_=pW[:, F2 + 1: F2 + 2], func=Exp, scale=ALPHA)
                w2col = work.tile([128, 1], dt, tag="w2col")
                nc.scalar.activation(out=w2col, in_=pW[:, F2: F2 + 1], func=Exp,
                                     scale=ALPHA - 1.0)
                pT3 = pstp.tile([128, 128], dt, tag="tp")
                nc.tensor.transpose(out=pT3[0:1, :], in_=w2col, identity=identb)
                nc.vector.tensor_copy(
                    out=w2r_sb[0:1, it * 128: (it + 1) * 128], in_=pT3[0:1, :])
            nc.sync.dma_start(out=gsend_t, in_=gsnd)

            if phases == "l1":
                zf = const.tile([128, F2], f32)
                nc.vector.memset(zf, 0.0)
                for rt in range(RT):
                    nc.sync.dma_start(out=out_d[rt * 128:(rt + 1) * 128, :],
                                      in_=zf)
                continue

            if dump:
                df = const.tile([128, 256], f32)
                if dump in ("z00", "num0"):
                    nc.vector.tensor_copy(out=df, in_=dmp)
                elif dump == "hloc0":
                    nc.vector.tensor_copy(out=df, in_=hlocT[:, 0:256])
                elif dump == "wb0":
                    nc.vector.tensor_copy(out=df, in_=wb[:, 0:256])
                elif dump == "wh0":
                    nc.vector.tensor_copy(out=df, in_=wh_sb[:, 0:256])
                elif dump == "quf0":
                    nc.vector.tensor_copy(out=df, in_=quf[:, 0:256])
                nc.sync.dma_start(
                    out=out_d[:, :].rearrange("(a p) f -> p a f", p=128),
                    in_=df[:, :].rearrange("p (a f) -> p a f", f=F2))
                continue
            # (layer-2 local projections inlined in the normalize loop)

            w2b = const.tile([128, R], dt)
            nc.gpsimd.partition_broadcast(out_ap=w2b, in_ap=w2r_sb[0:1, :])

            if collective:
                nc.gpsimd.collective_compute(
                    "AllGather", bypass,
                    replica_groups=[list(range(cfg.CORES))],
                    ins=[gsend_t.opt()], outs=[gfull_t.opt()],
                )
            gf_sb = const.tile([128, JT * PAY2], dt)
            if not collective:
                for c in range(cfg.CORES):
                    _e = [nc.sync, nc.scalar][c % 2]
                    _e.dma_start(
                        out=gfull_t[c * 128: (c + 1) * 128, :],
                        in_=gsend_t[:, :])
            for cp in range(2):
                nc.sync.dma_start(
                    out=gf_sb[:, cp * 4 * RT * PAY2: (cp + 1) * 4 * RT * PAY2]
                        .rearrange("p (c w) -> p c w", c=4),
                    in_=gfull_t[cp * 512: (cp + 1) * 512, :]
                        .rearrange("(c p) w -> p c w", p=128))
            qu2f = const.tile([128, JT * 2], f32)
            gf_r = gf_sb[:, :].rearrange("p (t c) -> p t c", c=PAY2)
            qu2f_r = qu2f[:, :].rearrange("p (t c) -> p t c", c=2)

            def qu2f_copy(c):
                nc.vector.tensor_copy(
                    out=qu2f_r[:, c * RT: (c + 1) * RT, :],
                    in_=gf_r[:, c * RT: (c + 1) * RT, F2 + 1: F2 + 3])

            # ---- layer 2 attention ------------------------------------
            psOt = psn.tile([128, HH], f32, tag="s0", name="psOt")
            for g in range(NG):
                qu2f_copy(2 * g)
                qu2f_copy(2 * g + 1)
                zr = work.tile([128, GRP * R], dt, tag="zr")
                for k in range(GRP):
                    t = g * GRP + k
                    nc.vector.tensor_scalar(
                        out=zr[:, k * R: (k + 1) * R],
                        in0=w2b,
                        scalar1=qu2f[:, 2 * t + 1: 2 * t + 2],
                        scalar2=qu2f[:, 2 * t: 2 * t + 1],
                        op0=mult, op1=mx,
                    )
                zg = wz.tile([128, GRP * R], dt, tag="zg")
                _sp = pool_num * R
                base_m = g * GRP * R
                nc.vector.tensor_tensor(
                    out=zg[:, 0: GRP * R - _sp], in0=zr[:, 0: GRP * R - _sp],
                    in1=km_sb[:, base_m: base_m + GRP * R - _sp], op=mult)
                if _sp:
                    nc.gpsimd.tensor_tensor(
                        out=zg[:, GRP * R - _sp:], in0=zr[:, GRP * R - _sp:],
                        in1=km_sb[:, base_m + GRP * R - _sp: base_m + GRP * R],
                        op=mult)
                for k in range(GRP):
                    t = g * GRP + k
                    for it in range(RT):
                        nc.tensor.matmul(
                            out=psOt[:, it * 128: it * 128 + F2 + 1],
                            lhsT=zg[:, k * R + it * 128: k * R + (it + 1) * 128],
                            rhs=gf_sb[:, t * PAY2: t * PAY2 + F2 + 1],
                            start=(t == 0 and it == 0), stop=(t == JT - 1),
                        )

            # ---- finalize: normalize, store ---------------------------
            for it in range(RT):
                rc = work.tile([128, 1], f32, tag="rc")
                nc.vector.reciprocal(out=rc, in_=psOt[:, it * 128 + F2: it * 128 + F2 + 1])
                of = work.tile([128, F2], f32, tag="of")
                nc.scalar.activation(out=of, in_=psOt[:, it * 128: it * 128 + F2],
                                     func=Copy, scale=rc)
                nc.sync.dma_start(
                    out=out_d[it * 128: (it + 1) * 128, :], in_=of
                )

    nc.compile()
    return nc


# --------------------------------------------------------------------------
# Host-side prep / sharding
# --------------------------------------------------------------------------

def host_prep(cfg: Cfg, g, inputs, W1, a1, W2, a2):
    N, C, H, HEADS, F2, R = cfg.N, cfg.C, cfg.H, cfg.HEADS, cfg.F2, cfg.R
    X = np.asarray(inputs, np.float32)
    W1 = np.asarray(W1, np.float32)
    a1 = np.asarray(a1, np.float32)
    W2 = np.asarray(W2, np.float32)
    a2 = np.asarray(a2, np.float32)

    def tile128(A):
        # [k*128, cols] row-major -> partition-major [128, k*cols]
        k = A.shape[0] // 128
        return np.ascontiguousarray(
            A.reshape(k, 128, A.shape[1]).transpose(1, 0, 2).reshape(128, -1)
        )

    XT = np.ascontiguousarray(X.T).astype(BF16)                       # [C, N]
    w1c = tile128(np.ascontiguousarray(
        W1.transpose(1, 0, 2).reshape(C, HEADS * H)).astype(BF16))
    # fused score projections: cols 0..3 = W1[h] @ a1_first (s1),
    # cols 4..7 = W1[h] @ a1_second (s2)
    ws = np.zeros((C, 8), np.float32)
    for h in range(HEADS):
        ws[:, h] = W1[h] @ a1[h][:H, 0]
        ws[:, 4 + h] = W1[h] @ a1[h][H:, 0]
    ws1 = tile128(ws.astype(BF16))
    # layer-2 weights with fused a2 projection columns
    F2p = F2 + 2
    w2f = np.zeros((HEADS * H, F2p), np.float32)
    w2f[:, 0:F2] = W2
    w2f[:, F2] = W2 @ a2[:F2, 0]
    w2f[:, F2 + 1] = W2 @ a2[F2:, 0]
    w2a = tile128(w2f.astype(BF16))
    ident = np.eye(128, dtype=BF16)

    adj = np.asarray(g) > 0
    in_maps = []
    for c in range(cfg.CORES):
        rows = slice(c * R, (c + 1) * R)
        km = np.where(adj[rows].T, MASK_K, 0.0).astype(BF16)          # [N, R]
        in_maps.append({
            "xtloc": tile128(np.ascontiguousarray(XT[:, rows])),
            "km": tile128(km),
            "w1c": w1c, "ws1": ws1, "w2a": w2a,
            "ident": ident,
        })
    return in_maps


_NC_CACHE = {}


def get_compiled(cfg: Cfg):
    nc = _NC_CACHE.get(cfg)
    if nc is None:
        nc = build_gat_nc(cfg)
        _NC_CACHE[cfg] = nc
    return nc


def kernel(g, inputs, W1, a1, W2, a2):
    cfg = FULL
    nc = get_compiled(cfg)
    in_maps = host_prep(cfg, g, inputs, W1, a1, W2, a2)
    res = run_bass_kernel_spmd(nc, in_maps, core_ids=list(range(cfg.CORES)))
    out = np.concatenate(
        [np.asarray(res.results[c]["out"], np.float32) for c in range(cfg.CORES)],
        axis=0,
    )
    return out
